# revision 35
# baseline (speedup 1.0000x reference)
"""SRP layer distributed Bass kernel for TRN2 (v23, ~568.5 us HW).

Math (full problem): out = Psi_c @ x.T @ x with Psi_c = Psi - rowmean(Psi).
  x [D, N] f32, Psi [O, N] f32, out [O, N] f32  (D=4096, N=8192, O=2048)

Distribution over 8 cores as a 4x2 grid: core c -> (i = c % 4: n-quarter,
j = c // 4: o-half). The host pre-centers Psi (global row-mean), pre-slices,
pre-transposes, and pre-casts to bf16, so the device does NOTHING but the
two GEMMs and the tmp AllReduce:

Per core (NL = N/4 = 2048, OL = O/2 = 1024):
  xT   [NL, D]  bf16  (x_i.T)        - mm1 stationary operand
  x    [D, NL]  bf16  (x_i)          - mm2 moving operand
  psiT [NL, OL] bf16  (Psi_c_ji.T)   - mm1 moving operand
  out  [OL, NL] f32

mm1: tmpT[d, o] = sum_n xT[n, d] * psiT[n, o]   (partial over local n)
     -> bf16 -> DRAM in 6 d-chunks (2,2,4,8,4,12 d-tiles), each
     AllReduce'd over the 4 cores of the same o-half as soon as it is
     ready (small leaders absorb the ~40us cross-core start skew).
mm2: out[o, n] = sum_d tmpT[d, o] * x[d, n], two kd-half passes so pass A
     (kd 0..15, AR chunks 0-3) runs while chunks 4-5 still AllReduce;
     pass B adds and streams the f32 result out.

Perf notes (hardware-measured; see git-less history v12->v23 in the
inline comments):
- Sustained MM pitch is 263 ns (512 cyc at 1.95 GHz: the package SW
  throttler holds k=13/16 under full-PE load; 2.4 GHz only in bursts),
  so the PE floor here is 2048 x 263 = 539 us. v23 measures ~568.5 us:
  ~8 us preamble+first-fill, ~10 us HBM-bound early feed stalls, ~6 us
  output drain + NEFF epilogue, remainder at-pitch matmul streaming.
- THE core scheduling hazard on this platform (cost 30-50 us per
  incident, v12-v22): Tile expresses waits as per-ring completion-sem
  lane thresholds, merges/elides them (ring-FIFO covering increments),
  and the pool-close barrier is a full DMA-queue quiesce. Any DMA or
  collective that (a) completes after mm1 ends and (b) shares a ring/
  lane horizon with something mm1's drain or pass A needs, stalls the
  PE for the difference. v23's rules:
    * Every pre-close AllReduce must COMPLETE before mm1 ends (chunk
      sizes tuned so AR4 ends ~283 us vs mm1 ~298 us); only the final
      3 MiB chunk's AR is emitted after the sb1 close (the quiesce then
      excludes it), triggering at ~mm1-end.
    * tmp_sb loads for chunks 0-4 ride the gpsimd SWDGE ring and are
      its FINAL DMAs, so pass-A waits can only retarget onto things
      that complete before mm1 ends. The last chunk's load goes to the
      scalar ring, anchored via a 4-element WAW write that depends on
      pass A's first out_part copy (the scheduler cannot hoist it).
    * Late x2b sets are first-use tiles in sb1's freed space (sb2 pool)
      instead of rotating over live buffers: rotation WARs get
      wait-coarsened by a whole ncn iteration (v22: 10 us stall).
- Bulk loads are single multi-dim DMAs (per xT chunk / psiT piece / tmp
  chunk / x2b half) over scalar+gpsimd+sync queues (DVE cannot issue
  DMAs; one HWDGE ring sustains only ~190 GB/s, and dt0 needs 5 MiB in
  its first ~19 us).
- A 16-MM zero-input warmup stream flips HAM (k=4/8 -> 8/8) before the
  first real matmul; real MMs otherwise run their first ~15 us at
  1.2 GHz. Dummy fill between early dt's does NOT help (feed-bound).
- Matmul operands slice a few BIG consolidated SBUF tiles: per-matmul
  semaphore waits otherwise break LDWEIGHTS pipelining (+45 ns/MM).
- t1s staging is 24 deep and mm1 holds 6 PSUM banks so mm1 coasts
  through the DMA blackout during each AllReduce's transfer phase.
- fp8/DoubleRow was evaluated and is numerically DEAD here: full-size
  CPU sim gives rel err 4.8e-2 (e4m3 both GEMMs), 3.0e-2/3.8e-2
  (mixed), 2.2e-2 (int8) vs the 2e-2 gate; bf16 pipeline = 2.7e-3.
  Strassen is throughput-dead: the C-assembly adds (~49 us DVE per
  GEMM) exceed the 12.5% PE saving (~34 us).
"""

from contextlib import ExitStack

import concourse.bacc as bacc
import concourse.mybir as mybir
import concourse.tile as tile

F32 = mybir.dt.float32
BF = mybir.dt.bfloat16


def build_srp_kernel(
    D=4096,
    NL=2048,
    OL=1024,
    n_cores=8,
    groups=((0, 1, 2, 3), (4, 5, 6, 7)),
):
    DT = D // 128    # 32 d-tiles (tmpT partition tiles / mm2 contraction)
    NT = NL // 128   # 16 n-tiles (mm1 contraction)
    OC = OL // 512   # 2  o-chunks (mm1 free cols)
    NCH = NL // 512  # 4  n-chunks (mm2 free cols)
    OT = OL // 128   # 8  o-tiles (mm2 output partition tiles)
    DC = D // 512    # 8  xT d-chunks (streamed)
    # AllReduce chunk sizes in d-tiles. Constraint discovered in v17/v18:
    # the sb1 pool-close barrier is a full DMA-queue quiesce, so it waits
    # for EVERY pre-close collective to drain — any AR still in flight
    # when mm1 ends stalls the PE for the difference. Chunks are therefore
    # sized so ARs 0-4 (6 MiB) complete inside mm1's ~290us (small leaders
    # absorb the ~40us cross-core skew), and only the last chunk's AR is
    # emitted after the close. Chunks 0..3 -> mm2 pass A, 4..5 -> pass B.
    CH_DT = (2, 2, 4, 8, 4, 12)
    CH_START = [sum(CH_DT[:i]) for i in range(len(CH_DT))]
    KH = DT // 2     # 16 kd per mm2 pass

    groups = [list(g) for g in groups]

    nc = bacc.Bacc("TRN2", target_bir_lowering=False, debug=False,
                   num_devices=n_cores)
    xT_ext = nc.dram_tensor("xT", [NL, D], BF, kind="ExternalInput")
    x_ext = nc.dram_tensor("x", [D, NL], BF, kind="ExternalInput")
    psiT_ext = nc.dram_tensor("psiT", [NL, OL], BF, kind="ExternalInput")
    out_ext = nc.dram_tensor("out", [OL, NL], F32, kind="ExternalOutput")

    # [p, nt, c] views: partition-first on both sides of every bulk DMA.
    psiT_r = psiT_ext.rearrange("(nt p) (oc c) -> oc p nt c", p=128, c=512)
    xT_r = xT_ext.rearrange("(nt p) (dc c) -> dc p nt c", p=128, c=512)
    x_r = x_ext.rearrange("(ph dq p) (ncn c) -> ph ncn p dq c",
                          dq=KH, p=128, c=512)

    with ExitStack() as stack:
        tc = stack.enter_context(tile.TileContext(nc))
        dram = stack.enter_context(tc.tile_pool(name="dram", bufs=1, space="DRAM"))
        ps = stack.enter_context(tc.tile_pool(name="ps", bufs=1, space="PSUM"))
        sbl = stack.enter_context(tc.tile_pool(name="sbl", bufs=1))

        tmp_in = [dram.tile([CH_DT[q] * 128, OL], BF, tag=f"tmp_in{q}", bufs=1,
                            name=f"tmp_in{q}") for q in range(len(CH_DT))]
        tmp_out = [dram.tile([CH_DT[q] * 128, OL], BF, tag=f"tmp_out{q}", bufs=1,
                             name=f"tmp_out{q}") for q in range(len(CH_DT))]

        tmp_sb = [sbl.tile([128, KH * OL], BF, tag="tmp_sb", bufs=2,
                           name=f"tmp_sb{p}") for p in range(2)]
        x2b = {}

        # ============ mm1 ============
        with tc.tile_pool(name="sb1", bufs=1) as sb1:
            # psiT split into two o-halves, each one big tile [128, NT*512]
            # (block nt at cols nt*512..). Four 4-nt pieces per half so the
            # first MMs start as soon as piece 0 lands; oc0 on scalar,
            # oc1 on gpsimd (idle until AR0, ~70us in).
            # PE clock warmup: HAM un-throttles (k=4/8 -> 8/8) only after
            # ~3.4us of sustained PE activity, so without this the first
            # ~15us of real matmuls run at 1.2 GHz. A zero-filled dummy
            # stream starting ~5us in (while the first loads are still in
            # flight) flips the clock before real data lands. Shares the
            # mm1 PSUM tag (WAW-only reuse - the group has no reader).
            warm = sb1.tile([128, 640], BF, tag="warm", bufs=1, name="warm")
            nc.vector.memset(warm[:, :], 0.0)
            wps = ps.tile([128, 512], F32, tag="mm1", bufs=6, name="warm_ps")
            for i in range(16):
                nc.tensor.matmul(wps[:], warm[:, 512:640], warm[:, 0:512],
                                 start=(i == 0), stop=(i == 15))

            psiT_sb = [sb1.tile([128, NT * 512], BF, tag=f"psiT{oc}", bufs=1,
                                name=f"psiT{oc}") for oc in range(OC)]
            for oc in range(OC):
                eng = nc.scalar if oc == 0 else nc.gpsimd
                dst3 = psiT_sb[oc][:, :].rearrange("p (nt c) -> p nt c", c=512)
                np_ = 8
                step = NT // np_
                for g in range(np_):
                    eng.dma_start(dst3[:, g * step:(g + 1) * step],
                                  psiT_r[oc, :, g * step:(g + 1) * step])

            # xT chunks: one big tile per 512-d-col chunk [128, NT*512],
            # rotating through 3 buffers, all on sync (DVE cannot issue
            # DMAs); chunk 0 in 4 pieces so MM0 starts at piece 0.
            xtc = {}

            def load_chunk(dc, eng, pieces=1):
                t = sb1.tile([128, NT * 512], BF, tag="xTc", bufs=3,
                             name=f"xTc{dc}")
                dst3 = t[:, :].rearrange("p (nt c) -> p nt c", c=512)
                step = NT // pieces
                for g in range(pieces):
                    eng.dma_start(dst3[:, g * step:(g + 1) * step],
                                  xT_r[dc, :, g * step:(g + 1) * step])
                xtc[dc] = t

            load_chunk(0, nc.sync, pieces=8)
            load_chunk(1, nc.sync, pieces=4)
            load_chunk(2, nc.sync, pieces=2)

            def emit_ar(q):
                # gpsimd carries the collectives: a collective blocks its
                # issuing queue until completion, so nothing time-critical
                # may queue behind one.
                nc.gpsimd.collective_compute(
                    "AllReduce", mybir.AluOpType.add,
                    replica_groups=groups,
                    ins=[tmp_in[q].opt()], outs=[tmp_out[q].opt()])

            def load_tmp_q(q, eng):
                # Chunks 0-4 ride the gpsimd SWDGE queue AND are that
                # ring's final DMAs: ring-FIFO covering-increment elision
                # can only retarget a consumer's wait onto a LATER ring
                # DMA, and everything on this ring completes before mm1
                # ends. The last chunk goes to the scalar HWDGE ring,
                # anchored behind a pass-A event.
                p = CH_START[q] // KH
                col0 = (CH_START[q] - p * KH) * OL
                src = tmp_out[q][:, :].rearrange("(dq p) o -> p dq o", p=128)
                dst = tmp_sb[p][:, col0:col0 + CH_DT[q] * OL].rearrange(
                    "p (dq o) -> p dq o", o=OL)
                eng.dma_start(dst, src)

            for dt in range(DT):
                dc = dt // 4
                if dt % 4 == 0 and dc + 3 < DC:
                    load_chunk(dc + 3, nc.sync)
                mm = [ps.tile([128, 512], F32, tag="mm1", bufs=6,
                              name=f"mm1_{dt}_{_oc}") for _oc in range(OC)]
                doff = (dt % 4) * 128
                for ntt in range(NT):
                    for oc in range(OC):
                        nc.tensor.matmul(
                            mm[oc][:],
                            xtc[dc][:, ntt * 512 + doff:ntt * 512 + doff + 128],
                            psiT_sb[oc][:, ntt * 512:(ntt + 1) * 512],
                            start=(ntt == 0), stop=(ntt == NT - 1))
                q = max(i for i in range(len(CH_DT)) if CH_START[i] <= dt)
                dq = dt - CH_START[q]
                for oc in range(OC):
                    st = sb1.tile([128, 512], BF, tag="t1s", bufs=24,
                                  name=f"t1s{dt}_{oc}")
                    nc.vector.tensor_copy(st[:], mm[oc][:])
                    nc.scalar.dma_start(
                        tmp_in[q][dq * 128:(dq + 1) * 128,
                                  oc * 512:(oc + 1) * 512],
                        st[:])
                if dq == CH_DT[q] - 1 and q < len(CH_DT) - 1:
                    emit_ar(q)
                    # its tmp_sb load slots between this AR's trigger and
                    # the next in the gpsimd FIFO; the enqueue waits only
                    # on this AR's completion and the transfer overlaps
                    # the next collective.
                    load_tmp_q(q, nc.gpsimd)

        # The final chunk's AllReduce completes well after mm1 ends, so it
        # must be emitted AFTER the sb1 close: the close barrier's queue
        # quiesce then excludes it. Its trigger sits behind the close
        # barrier on gpsimd (~mm1 end) — pass B consumes it ~130us later.
        emit_ar(len(CH_DT) - 1)

        def load_x2b(p, ncn, pool, tag):
            t = pool.tile([128, KH * 512], BF, tag=tag, bufs=2,
                          name=f"x2b{p}_{ncn}")
            d3 = t[:, :].rearrange("p (dq c) -> p dq c", c=512)
            # two half-DMAs: the consumer's ring-FIFO covering increment
            # then lands on this set's own second half, not on the NEXT
            # x2b load (which only starts after another full ncn of MMs —
            # that retargeting cost a 7.8us PE stall at the ncn1->ncn2
            # boundary when the load was a single DMA).
            h = KH // 2
            nc.sync.dma_start(d3[:, :h], x_r[p, ncn, :, :h])
            nc.sync.dma_start(d3[:, h:], x_r[p, ncn, :, h:])
            x2b[(p, ncn)] = t

        # First two x2b sets load right after the close (fresh buffers).
        load_x2b(0, 0, sbl, "x2b")
        load_x2b(0, 1, sbl, "x2b")

        # ============ mm2 ============
        with tc.tile_pool(name="sb2", bufs=1) as sb2:
            out_part = [sb2.tile([128, NL], F32, tag="out_part", bufs=OT,
                                 name=f"out_part{ot}") for ot in range(OT)]
            # The LATE pass-A sets (0,2)/(0,3) and the last pass-B sets go
            # in sb1's freed space via the sb2 pool (tag x2c): they are
            # then first-use tiles with NO rotation WAR, so their loads run
            # right after the pool-open barrier (~mm1 end) instead of
            # being gated on a whole ncn of matmuls (v22: a wait-coarsened
            # rotation dep made x2b(0,2) land 10us after ncn2 needed it).
            # Pass-B sets (1,0)/(1,1) rotate over the sbl buffers whose
            # readers (pass-A ncn0/ncn1) finish >60us before they are
            # consumed — coarsening-proof.
            load_x2b(0, 2, sb2, "x2c")
            load_x2b(0, 3, sb2, "x2c")
            load_x2b(1, 0, sbl, "x2b")
            load_x2b(1, 1, sbl, "x2b")
            load_x2b(1, 2, sb2, "x2c")
            load_x2b(1, 3, sb2, "x2c")
            for p in range(2):
                for ncn in range(NCH):
                    for ot in range(OT):
                        mmo = ps.tile([128, 512], F32, tag="mm2", bufs=2,
                                      name=f"mm2_{p}_{ncn}_{ot}")
                        for dq in range(KH):
                            nc.tensor.matmul(
                                mmo[:],
                                tmp_sb[p][:, dq * OL + ot * 128:
                                          dq * OL + (ot + 1) * 128],
                                x2b[(p, ncn)][:, dq * 512:(dq + 1) * 512],
                                start=(dq == 0), stop=(dq == KH - 1))
                        if p == 0:
                            nc.vector.tensor_copy(
                                out_part[ot][:, ncn * 512:(ncn + 1) * 512],
                                mmo[:])
                            if ncn == 0 and ot == 0:
                                # Anchor: a 4-element WAW write into the
                                # final chunk's tmp_sb[1] region that
                                # depends on pass A's first out_part copy,
                                # so the scheduler cannot hoist the
                                # AR-gated last tmp load (which overwrites
                                # it) into mm1's drain window, where its
                                # ring-lane slot would re-create the
                                # transition bubble. The DMA overwrites
                                # these 4 garbage elements.
                                qlast = len(CH_DT) - 1
                                c0 = (CH_START[qlast] - KH) * OL
                                nc.vector.tensor_copy(
                                    tmp_sb[1][:, c0:c0 + 4],
                                    out_part[0][:, 0:4])
                                load_tmp_q(qlast, nc.scalar)
                        else:
                            ost = sb2.tile([128, 512], F32, tag="ost", bufs=4,
                                           name=f"ost{ot}_{ncn}")
                            nc.vector.tensor_tensor(
                                ost[:], mmo[:],
                                out_part[ot][:, ncn * 512:(ncn + 1) * 512],
                                op=mybir.AluOpType.add)
                            # alternate stage-out queues so the final ncn
                            # group's drain is split across two engines
                            eng = nc.scalar if ot % 2 == 0 else nc.sync
                            eng.dma_start(
                                out_ext[ot * 128:(ot + 1) * 128,
                                        ncn * 512:(ncn + 1) * 512],
                                ost[:])
    nc.compile()
    return nc


def make_in_maps(x, Psi, n_cores=8, NL=2048, OL=1024):
    """Shard full f32 inputs for the 4x2 grid with host-side prep:
    center Psi with the global row-mean, slice, transpose, cast bf16."""
    import numpy as np
    import ml_dtypes
    bf16 = ml_dtypes.bfloat16

    Psi_c = (Psi.astype(np.float64)
             - Psi.mean(axis=1, dtype=np.float64, keepdims=True))
    in_maps = []
    for c in range(n_cores):
        i, j = c % 4, c // 4
        xs = x[:, i * NL:(i + 1) * NL].astype(np.float32)
        ps_ = Psi_c[j * OL:(j + 1) * OL, i * NL:(i + 1) * NL]
        in_maps.append({
            "x": np.ascontiguousarray(xs).astype(bf16),
            "xT": np.ascontiguousarray(xs.T).astype(bf16),
            "psiT": np.ascontiguousarray(ps_.T).astype(bf16),
        })
    return in_maps


# ---------------- harness-facing wrapper ----------------
import numpy as np

_NC_CACHE = {}

D_FULL, N_FULL, O_FULL = 4096, 8192, 2048
NL_, OL_ = 2048, 1024
N_CORES = 8
GROUPS = ((0, 1, 2, 3), (4, 5, 6, 7))


def _get_nc():
    if "nc" not in _NC_CACHE:
        _NC_CACHE["nc"] = build_srp_kernel(
            D=D_FULL, NL=NL_, OL=OL_, n_cores=N_CORES, groups=GROUPS)
    return _NC_CACHE["nc"]


def kernel(x, Psi):
    """out = (Psi - rowmean(Psi)) @ x.T @ x on 8 TRN2 NeuronCores."""
    from concourse.bass_utils import run_bass_kernel_spmd
    x = np.asarray(x, dtype=np.float32)
    Psi = np.asarray(Psi, dtype=np.float32)
    assert x.shape == (D_FULL, N_FULL) and Psi.shape == (O_FULL, N_FULL)
    nc = _get_nc()
    in_maps = make_in_maps(x, Psi, n_cores=N_CORES, NL=NL_, OL=OL_)
    res = run_bass_kernel_spmd(nc, in_maps, core_ids=list(range(N_CORES)))
    out = np.empty((O_FULL, N_FULL), dtype=np.float32)
    for c in range(N_CORES):
        i, j = c % 4, c // 4
        out[j * OL_:(j + 1) * OL_, i * NL_:(i + 1) * NL_] = res.results[c]["out"]
    return out
